# revision 1
# baseline (speedup 1.0000x reference)
"""Trainium2 Bass kernel for nn_GruAgent (GRU + actor/critic MLP heads).

Strategy (per spec sharding hint): data-parallel over the env dim B across
8 NeuronCores (64 envs/core), weights replicated.  Inside each core the
recurrence runs in a transposed layout [channels, envs] so the per-step
hidden matmuls feed the tensor engine directly; the input projection (gi),
the actor/critic MLP and all transposes/DMA are overlapped with the
sequential GRU chain.

Self-contained: hardcodes all shapes; only depends on the platform's
concourse (Bass) library.
"""

import os
import sys

import numpy as np

for _p in ("/opt/trn_rl_repo", os.path.expanduser("~/.axon_site/_ro/trn_rl_repo")):
    if os.path.isdir(_p) and _p not in sys.path:
        sys.path.insert(0, _p)
        break

import concourse.bass as bass
import concourse.mybir as mybir
import concourse.tile as tile
from concourse import bacc
from concourse.masks import make_identity

T, B, OBS, H, A, L = 512, 512, 64, 64, 6, 64
N_CORES = 8
BL = B // N_CORES          # 64 envs per core
GS = 8                     # timesteps per group
COLS = GS * BL             # 512 columns (free dim) per group
H3 = 3 * H

F32 = mybir.dt.float32
AF = mybir.ActivationFunctionType
ALU = mybir.AluOpType

WEIGHT_KEYS = [
    "w_ih", "w_hh", "b_ih", "b_hh",
    "aw1", "ab1", "aw2", "ab2", "aw3", "ab3",
    "cw1", "cb1", "cw2", "cb2", "cw3", "cb3",
]


def build(nc, t_loc=T):
    """Emit the full per-core kernel into `nc` (a Bacc instance)."""
    from contextlib import ExitStack

    assert t_loc % GS == 0
    ng = t_loc // GS

    x_d = nc.dram_tensor("x", [t_loc, BL, OBS], F32, kind="ExternalInput")
    done_d = nc.dram_tensor("done", [t_loc, BL], F32, kind="ExternalInput")
    h0_d = nc.dram_tensor("h0", [BL, H], F32, kind="ExternalInput")
    wih_d = nc.dram_tensor("w_ih", [H3, OBS], F32, kind="ExternalInput")
    whh_d = nc.dram_tensor("w_hh", [H3, H], F32, kind="ExternalInput")
    bih_d = nc.dram_tensor("b_ih", [H3], F32, kind="ExternalInput")
    bhh_d = nc.dram_tensor("b_hh", [H3], F32, kind="ExternalInput")
    aw1_d = nc.dram_tensor("aw1", [L, H + OBS], F32, kind="ExternalInput")
    ab1_d = nc.dram_tensor("ab1", [L], F32, kind="ExternalInput")
    aw2_d = nc.dram_tensor("aw2", [L, L], F32, kind="ExternalInput")
    ab2_d = nc.dram_tensor("ab2", [L], F32, kind="ExternalInput")
    aw3_d = nc.dram_tensor("aw3", [A, L], F32, kind="ExternalInput")
    ab3_d = nc.dram_tensor("ab3", [A], F32, kind="ExternalInput")
    cw1_d = nc.dram_tensor("cw1", [L, H + OBS], F32, kind="ExternalInput")
    cb1_d = nc.dram_tensor("cb1", [L], F32, kind="ExternalInput")
    cw2_d = nc.dram_tensor("cw2", [L, L], F32, kind="ExternalInput")
    cb2_d = nc.dram_tensor("cb2", [L], F32, kind="ExternalInput")
    cw3_d = nc.dram_tensor("cw3", [1, L], F32, kind="ExternalInput")
    cb3_d = nc.dram_tensor("cb3", [1], F32, kind="ExternalInput")
    out_d = nc.dram_tensor("out", [t_loc, BL, A + 1], F32, kind="ExternalOutput")

    with tile.TileContext(nc) as tc, ExitStack() as ctx:
        wp = ctx.enter_context(tc.tile_pool(name="wp", bufs=1))
        ldp = ctx.enter_context(tc.tile_pool(name="ldp", bufs=2))
        catp = ctx.enter_context(tc.tile_pool(name="catp", bufs=3))
        xnp = ctx.enter_context(tc.tile_pool(name="xnp", bufs=2))
        drp = ctx.enter_context(tc.tile_pool(name="drp", bufs=2))
        mbp = ctx.enter_context(tc.tile_pool(name="mbp", bufs=3))
        small = ctx.enter_context(tc.tile_pool(name="small", bufs=3))
        tmlp = ctx.enter_context(tc.tile_pool(name="tmlp", bufs=2))
        onp = ctx.enter_context(tc.tile_pool(name="onp", bufs=2))

        przp = ctx.enter_context(tc.tile_pool(name="przp", bufs=2, space="PSUM"))
        pginp = ctx.enter_context(tc.tile_pool(name="pginp", bufs=2, space="PSUM"))
        pghnp = ctx.enter_context(tc.tile_pool(name="pghnp", bufs=2, space="PSUM"))
        pmisc = ctx.enter_context(tc.tile_pool(name="pmisc", bufs=2, space="PSUM"))

        ident = wp.tile([128, 128], F32, tag="ident")
        make_identity(nc, ident[:])

        def load_transposed(dram_ap, rows, cols, tag):
            """dram [rows, cols] -> sbuf tile [cols, rows] (features on partitions)."""
            dst = wp.tile([cols, rows], F32, tag=tag)
            r0 = 0
            while r0 < rows:
                rr = min(128, rows - r0)
                tmp = ldp.tile([128, 128], F32, tag="wtmp")
                nc.sync.dma_start(tmp[:rr, :cols], dram_ap[r0:r0 + rr, :])
                pt = pmisc.tile([128, COLS], F32, tag="pm")
                nc.tensor.transpose(pt[:cols, :rr], tmp[:rr, :cols], ident[:rr, :rr])
                nc.scalar.copy(dst[:, r0:r0 + rr], pt[:cols, :rr])
                r0 += rr
            return dst

        def load_col(dram_1d, n, tag, off=0, dst=None, dst_off=0):
            if dst is None:
                dst = wp.tile([max(n + dst_off, 1), 1], F32, tag=tag)
            nc.sync.dma_start(
                dst[dst_off:dst_off + n, :],
                dram_1d[off:off + n].rearrange("p -> p ()"),
            )
            return dst

        # --- weights / constants preprocessing (runs once, overlapped) ---
        w_ihT = load_transposed(wih_d[:], H3, OBS, "wihT")    # [64, 192]
        w_hhT = load_transposed(whh_d[:], H3, H, "whhT")      # [64, 192]
        h0T = load_transposed(h0_d[:], BL, H, "h0T")          # [64, 64] (h x b)

        lhsT1h = wp.tile([64, 128], F32, tag="lhsT1h")
        lhsT1x = wp.tile([64, 128], F32, tag="lhsT1x")
        for src, c0 in ((aw1_d, 0), (cw1_d, 64)):
            tmp = ldp.tile([128, 128], F32, tag="wtmp")
            nc.sync.dma_start(tmp[:L, :H + OBS], src[:, :])
            pt = pmisc.tile([128, COLS], F32, tag="pm")
            nc.tensor.transpose(pt[:H, :L], tmp[:L, 0:H], ident[:L, :L])
            nc.tensor.transpose(pt[:OBS, 128:128 + L], tmp[:L, H:H + OBS], ident[:L, :L])
            nc.scalar.copy(lhsT1h[:, c0:c0 + L], pt[:H, :L])
            nc.scalar.copy(lhsT1x[:, c0:c0 + L], pt[:OBS, 128:128 + L])

        lhsT2 = wp.tile([128, 128], F32, tag="lhsT2")
        nc.vector.memset(lhsT2[:], 0.0)
        for src, o in ((aw2_d, 0), (cw2_d, 64)):
            tmp = ldp.tile([128, 128], F32, tag="wtmp")
            nc.sync.dma_start(tmp[:L, :L], src[:, :])
            pt = pmisc.tile([128, COLS], F32, tag="pm")
            nc.tensor.transpose(pt[:L, :L], tmp[:L, :L], ident[:L, :L])
            nc.scalar.copy(lhsT2[o:o + L, o:o + L], pt[:L, :L])

        lhsT3 = wp.tile([128, A + 1], F32, tag="lhsT3")
        nc.vector.memset(lhsT3[:], 0.0)
        tmp = ldp.tile([128, 128], F32, tag="wtmp")
        nc.sync.dma_start(tmp[:A, :L], aw3_d[:, :])
        pt = pmisc.tile([128, COLS], F32, tag="pm")
        nc.tensor.transpose(pt[:L, :A], tmp[:A, :L], ident[:A, :A])
        nc.scalar.copy(lhsT3[:L, :A], pt[:L, :A])
        tmp = ldp.tile([128, 128], F32, tag="wtmp")
        nc.sync.dma_start(tmp[:1, :L], cw3_d[:, :])
        pt = pmisc.tile([128, COLS], F32, tag="pm")
        nc.tensor.transpose(pt[:L, :1], tmp[:1, :L], ident[:1, :1])
        nc.scalar.copy(lhsT3[64:64 + L, A:A + 1], pt[:L, :1])

        # biases
        bihc = load_col(bih_d, 128, "bihc")                   # b_ih[0:128]
        bhhc = load_col(bhh_d, 128, "bhhc")
        bias_r = wp.tile([64, 1], F32, tag="bias_r")
        nc.vector.tensor_add(bias_r[:], bihc[0:64, :], bhhc[0:64, :])
        bias_z = wp.tile([64, 1], F32, tag="bias_z")
        bihz = load_col(bih_d, 64, "bihz", off=64)
        bhhz = load_col(bhh_d, 64, "bhhz", off=64)
        nc.vector.tensor_add(bias_z[:], bihz[:], bhhz[:])
        negbz = wp.tile([64, 1], F32, tag="negbz")
        nc.vector.tensor_scalar_mul(negbz[:], bias_z[:], -1.0)
        b_ihn = load_col(bih_d, H, "b_ihn", off=128)          # [64,1]
        b_hhn = load_col(bhh_d, H, "b_hhn", off=128)          # [64,1]

        bias1 = wp.tile([128, 1], F32, tag="bias1")
        load_col(ab1_d, L, "bias1", dst=bias1, dst_off=0)
        load_col(cb1_d, L, "bias1", dst=bias1, dst_off=64)
        bias2 = wp.tile([128, 1], F32, tag="bias2")
        load_col(ab2_d, L, "bias2", dst=bias2, dst_off=0)
        load_col(cb2_d, L, "bias2", dst=bias2, dst_off=64)
        bias3 = wp.tile([A + 1, 1], F32, tag="bias3")
        load_col(ab3_d, A, "bias3", dst=bias3, dst_off=0)
        load_col(cb3_d, 1, "bias3", dst=bias3, dst_off=A)

        ones_row = wp.tile([1, BL], F32, tag="ones_row")
        nc.vector.memset(ones_row[:], 1.0)

        # --- steady-state group bodies ---
        def bulk(g):
            """x load + transpose, done -> reset-mask, gi preloads for group g."""
            hs = catp.tile([64, COLS], F32, tag="hs")
            xT = catp.tile([64, COLS], F32, tag="xT")
            xn = xnp.tile([128, GS // 2, OBS], F32, tag="xn")
            nc.sync.dma_start(
                xn[:],
                x_d[g * GS:(g + 1) * GS].rearrange("(k ph) b f -> (ph b) k f", ph=2),
            )
            ptx = pmisc.tile([128, COLS], F32, tag="pm")
            for k in range(GS // 2):
                nc.tensor.transpose(
                    ptx[:OBS, k * 128:(k + 1) * 128], xn[:, k, :], ident[:, :]
                )
            nc.scalar.copy(xT[:], ptx[:OBS, :])

            dr = drp.tile([1, COLS], F32, tag="dr")
            nc.sync.dma_start(
                dr[:], done_d[g * GS:(g + 1) * GS].rearrange("t b -> () (t b)")
            )
            pmb = pmisc.tile([128, COLS], F32, tag="pm")
            nc.tensor.matmul(pmb[:BL, :], ones_row[:], dr[:], start=True, stop=True)
            mb = mbp.tile([BL, COLS], F32, tag="mb")
            nc.scalar.activation(mb[:], pmb[:BL, :], AF.Identity, scale=-1.0, bias=1.0)

            prz = przp.tile([128, COLS], F32, tag="prz")
            nc.tensor.matmul(
                prz[:], w_ihT[:, 0:128], xT[:],
                start=True, stop=False, skip_group_check=True,
            )
            pgin = pginp.tile([BL, COLS], F32, tag="pgin")
            nc.tensor.matmul(
                pgin[:], w_ihT[:, 128:H3], xT[:], start=True, stop=True
            )
            return dict(hs=hs, xT=xT, mb=mb, prz=prz, pgin=pgin)

        state = {}

        def chain(g, refs, refs_next):
            prz, pgin, mb, hs = refs["prz"], refs["pgin"], refs["mb"], refs["hs"]
            for s in range(GS):
                t = g * GS + s
                cs = bass.ts(s, BL)
                mh = state["mh"]
                pghn = pghnp.tile([BL, BL], F32, tag="pghn")
                nc.tensor.matmul(
                    pghn[:], w_hhT[:, 128:H3], mh[:], start=True, stop=True
                )
                nc.tensor.matmul(
                    prz[:, cs], w_hhT[:, 0:128], mh[:],
                    start=False, stop=(s == GS - 1), skip_group_check=True,
                )
                r_t = small.tile([BL, BL], F32, tag="r_t")
                nc.scalar.activation(r_t[:], prz[0:64, cs], AF.Sigmoid, bias=bias_r[:])
                z_t = small.tile([BL, BL], F32, tag="z_t")
                nc.scalar.activation(z_t[:], prz[64:128, cs], AF.Sigmoid, bias=bias_z[:])
                u = small.tile([BL, BL], F32, tag="u")
                nc.scalar.activation(
                    u[:], prz[64:128, cs], AF.Sigmoid, scale=-1.0, bias=negbz[:]
                )
                zm = small.tile([BL, BL], F32, tag="zm")
                nc.gpsimd.tensor_mul(zm[:], z_t[:], mh[:])
                p = small.tile([BL, BL], F32, tag="p")
                nc.vector.scalar_tensor_tensor(
                    p[:], pghn[:], b_hhn[:], r_t[:], ALU.add, ALU.mult
                )
                q = small.tile([BL, BL], F32, tag="q")
                nc.vector.tensor_add(q[:], p[:], pgin[:, cs])
                n = small.tile([BL, BL], F32, tag="n")
                nc.scalar.activation(n[:], q[:], AF.Tanh, bias=b_ihn[:])
                v = small.tile([BL, BL], F32, tag="v")
                nc.vector.tensor_mul(v[:], n[:], u[:])
                nc.vector.tensor_add(hs[:, cs], v[:], zm[:])
                if t < t_loc - 1:
                    mh2 = small.tile([BL, BL], F32, tag="mh")
                    if s == GS - 1:
                        mbn = refs_next["mb"][:, 0:BL]
                    else:
                        mbn = mb[:, bass.ts(s + 1, BL)]
                    nc.vector.tensor_mul(mh2[:], hs[:, cs], mbn)
                    state["mh"] = mh2

        def head(g, refs):
            hs, xT = refs["hs"], refs["xT"]
            p1 = pmisc.tile([128, COLS], F32, tag="pm")
            nc.tensor.matmul(p1[:], lhsT1h[:], hs[:], start=True, stop=False,
                             skip_group_check=True)
            nc.tensor.matmul(p1[:], lhsT1x[:], xT[:], start=False, stop=True,
                             skip_group_check=True)
            t1 = tmlp.tile([128, COLS], F32, tag="t1")
            nc.scalar.activation(t1[:], p1[:], AF.Tanh, bias=bias1[:])
            p2 = pmisc.tile([128, COLS], F32, tag="pm")
            nc.tensor.matmul(p2[:], lhsT2[:], t1[:], start=True, stop=True)
            t2 = tmlp.tile([128, COLS], F32, tag="t2")
            nc.scalar.activation(t2[:], p2[:], AF.Tanh, bias=bias2[:])
            p3 = pmisc.tile([128, COLS], F32, tag="pm")
            nc.tensor.matmul(p3[:A + 1, :], lhsT3[:], t2[:], start=True, stop=True)
            o7 = tmlp.tile([A + 1, COLS], F32, tag="o7")
            nc.scalar.activation(o7[:], p3[:A + 1, :], AF.Identity, bias=bias3[:])

            po = pmisc.tile([128, GS // 2, A + 1], F32, tag="pm")
            for k in range(GS // 2):
                nc.tensor.transpose(
                    po[:, k, :], o7[:, k * 128:(k + 1) * 128], ident[:A + 1, :A + 1]
                )
            on = onp.tile([128, GS // 2, A + 1], F32, tag="on")
            nc.vector.tensor_copy(on[:], po[:])
            nc.sync.dma_start(
                out_d[g * GS:(g + 1) * GS].rearrange("(k ph) b j -> (ph b) k j", ph=2),
                on[:],
            )

        refs = bulk(0)
        mh0 = small.tile([BL, BL], F32, tag="mh")
        nc.vector.tensor_mul(mh0[:], h0T[:], refs["mb"][:, 0:BL])
        state["mh"] = mh0
        for g in range(1, ng):
            refs_next = bulk(g)
            chain(g - 1, refs, refs_next)
            head(g - 1, refs)
            refs = refs_next
        chain(ng - 1, refs, None)
        head(ng - 1, refs)

    return nc


_BUILT = {}


def get_built(t_loc=T):
    if t_loc not in _BUILT:
        nc = bacc.Bacc(None, target_bir_lowering=False)
        build(nc, t_loc)
        nc.compile()
        _BUILT[t_loc] = nc
    return _BUILT[t_loc]


def shard_inputs(inputs, t_loc=T):
    """Full inputs dict -> list of 8 per-core input maps."""
    x = np.ascontiguousarray(np.asarray(inputs["x"], np.float32)).reshape(t_loc, B, OBS)
    done = np.ascontiguousarray(np.asarray(inputs["done"], np.float32)).reshape(t_loc, B)
    h0 = np.ascontiguousarray(np.asarray(inputs["gru_state"], np.float32)).reshape(B, H)
    common = {
        k: np.ascontiguousarray(np.asarray(inputs[k], np.float32))
        for k in WEIGHT_KEYS
    }
    in_maps = []
    for c in range(N_CORES):
        sl = slice(c * BL, (c + 1) * BL)
        m = dict(common)
        m["x"] = np.ascontiguousarray(x[:, sl, :])
        m["done"] = np.ascontiguousarray(done[:, sl])
        m["h0"] = np.ascontiguousarray(h0[sl, :])
        in_maps.append(m)
    return in_maps


def assemble_output(per_core_outs, t_loc=T):
    outs = [np.asarray(o, np.float32).reshape(t_loc, BL, A + 1) for o in per_core_outs]
    full = np.stack(outs, axis=1).reshape(t_loc, B, A + 1)
    return np.ascontiguousarray(full.reshape(t_loc * B, A + 1))


def run_on_hw(inputs, t_loc=T, trace=False, **kw):
    from concourse.bass_utils import run_bass_kernel_spmd

    nc = get_built(t_loc)
    in_maps = shard_inputs(inputs, t_loc)
    res = run_bass_kernel_spmd(
        nc, in_maps, core_ids=list(range(N_CORES)), trace=trace, **kw
    )
    out = assemble_output([r["out"] for r in res.results], t_loc)
    return out, res


def kernel(**inputs):
    out, _ = run_on_hw(inputs)
    return out

